# revision 1
# baseline (speedup 1.0000x reference)
"""Grouped MLP (MoE expert-parallel) Trainium2 kernel.

Problem: hidden_states [131072, 1024] f32, 8 experts each owning a contiguous
16384-token block; per expert: SwiGLU MLP with gate_up [1024, 1024] and
down [512, 1024].

Sharding: expert-parallel — core e computes expert e's token block entirely
locally (no collectives). Inputs are sliced host-side, outputs concatenated.

Per-core kernel (per 128-token tile):
  - load x tile [128, 1024] f32 (HWDGE)
  - PE-transpose 8x [128,128] -> xT (f32r, rounded during PSUM->SBUF copy)
  - mm1: PSUM[128t, 512f] x2 accumulating over 8 h-slices;
         lhsT = xT[:,k,:] (stationary), rhs = W1[k, f-chunk] (f32r, resident)
  - SwiGLU: silu(gate) on ACT, * up on DVE (f32)
  - PE-transpose 4x -> hT (f32r)
  - mm2: PSUM[128t, 512]x2 over 4 i-slices with W2 resident
  - copy PSUM -> SBUF f32 out tile, DMA store (natural [tokens, H] layout)

float32r gives full PE rate (1 cyc/row at N=512) at ~1.5e-4 relative error.
"""

import numpy as np

E = 8
H = 1024
I = 512
T_PER_CORE = 16384
N_CORES = 8

_cache = {}


def _build_nc(n_tiles):
    import concourse.mybir as mybir
    import concourse.tile as tile
    from concourse import bacc
    from concourse.masks import make_identity

    f32 = mybir.dt.float32
    f32r = mybir.dt.float32r

    nc = bacc.Bacc(None, target_bir_lowering=False)
    n_tok = n_tiles * 128
    x = nc.dram_tensor("x", [n_tok, H], f32, kind="ExternalInput")
    w1 = nc.dram_tensor("w1", [H, 2 * I], f32, kind="ExternalInput")
    w2 = nc.dram_tensor("w2", [I, H], f32, kind="ExternalInput")
    out = nc.dram_tensor("out", [n_tok, H], f32, kind="ExternalOutput")

    with tile.TileContext(nc) as tc:
        with (
            tc.tile_pool(name="const", bufs=1) as const,
            tc.tile_pool(name="xin", bufs=4) as xin,
            tc.tile_pool(name="xtp", bufs=3) as xtp,
            tc.tile_pool(name="actp", bufs=4) as actp,
            tc.tile_pool(name="htp", bufs=3) as htp,
            tc.tile_pool(name="outp", bufs=3) as outp,
            tc.tile_pool(name="tp_ps", bufs=2, space="PSUM") as tp_ps_pool,
            tc.tile_pool(name="mm1_ps", bufs=4, space="PSUM") as mm1_ps_pool,
            tc.tile_pool(name="mm2_ps", bufs=2, space="PSUM") as mm2_ps_pool,
        ):
            # Resident weights, rounded to f32r during the load DMA (SWDGE cast).
            w1_sb = const.tile([128, H // 128, 2 * I], f32r)
            nc.gpsimd.dma_start(w1_sb[:], w1.ap().rearrange("(ho p) f -> p ho f", p=128))
            w2_sb = const.tile([128, I // 128, H], f32r)
            nc.gpsimd.dma_start(w2_sb[:], w2.ap().rearrange("(io p) f -> p io f", p=128))
            ident = const.tile([128, 128], f32)
            make_identity(nc, ident)

            # Software-pipelined emission. Per iteration i the PE stream is
            #   xtrans_i, mm1_{i-1}, mm2_{i-2}, htrans_{i-1}
            # so the SwiGLU chain of tile i-1 hides under mm2_{i-2}.
            xT_d, mm1_d, h_d, hT_d = {}, {}, {}, {}

            def stage_load_transpose(t):
                x_t = xin.tile([128, H], f32, tag="x")
                nc.sync.dma_start(x_t[:], x.ap()[t * 128 : (t + 1) * 128, :])
                xT = xtp.tile([128, H // 128, 128], f32r, tag="xT")
                for g in range(2):
                    tp_ps = tp_ps_pool.tile([128, 4, 128], f32, tag="tp")
                    for j in range(4):
                        k = g * 4 + j
                        nc.tensor.transpose(
                            tp_ps[:, j, :], x_t[:, k * 128 : (k + 1) * 128], ident
                        )
                    nc.scalar.copy(xT[:, g * 4 : (g + 1) * 4, :], tp_ps[:])
                xT_d[t] = xT

            def stage_mm1(t):
                xT = xT_d.pop(t)
                ps_pair = []
                for f in range(2):
                    ps = mm1_ps_pool.tile([128, 512], f32, tag="mm1")
                    for k in range(H // 128):
                        nc.tensor.matmul(
                            ps[:],
                            xT[:, k, :],
                            w1_sb[:, k, f * 512 : (f + 1) * 512],
                            start=(k == 0),
                            stop=(k == H // 128 - 1),
                        )
                    ps_pair.append(ps)
                mm1_d[t] = ps_pair

            def stage_swiglu(t):
                gate_ps, up_ps = mm1_d.pop(t)
                s = actp.tile([128, 512], f32, tag="s")
                nc.scalar.activation(
                    s[:], gate_ps[:], mybir.ActivationFunctionType.Sigmoid
                )
                t1 = actp.tile([128, 512], f32, tag="t1")
                nc.vector.tensor_mul(t1[:], s[:], up_ps[:])
                h = actp.tile([128, 512], f32, tag="h")
                nc.vector.tensor_mul(h[:], t1[:], gate_ps[:])
                h_d[t] = h

            def stage_htrans(t):
                h = h_d.pop(t)
                hT = htp.tile([128, I // 128, 128], f32r, tag="hT")
                tp_ps = tp_ps_pool.tile([128, 4, 128], f32, tag="tp")
                for k in range(4):
                    nc.tensor.transpose(
                        tp_ps[:, k, :], h[:, k * 128 : (k + 1) * 128], ident
                    )
                nc.scalar.copy(hT[:], tp_ps[:])
                hT_d[t] = hT

            def stage_mm2_store(t):
                hT = hT_d.pop(t)
                o_t = outp.tile([128, H], f32, tag="o")
                for f in range(2):
                    ps2 = mm2_ps_pool.tile([128, 512], f32, tag="mm2")
                    for k in range(I // 128):
                        nc.tensor.matmul(
                            ps2[:],
                            hT[:, k, :],
                            w2_sb[:, k, f * 512 : (f + 1) * 512],
                            start=(k == 0),
                            stop=(k == I // 128 - 1),
                        )
                    nc.vector.tensor_copy(o_t[:, f * 512 : (f + 1) * 512], ps2[:])
                nc.sync.dma_start(out.ap()[t * 128 : (t + 1) * 128, :], o_t[:])

            for i in range(n_tiles + 2):
                if i < n_tiles:
                    stage_load_transpose(i)
                if 1 <= i <= n_tiles:
                    stage_mm1(i - 1)
                    stage_swiglu(i - 1)
                if 2 <= i <= n_tiles + 1:
                    stage_mm2_store(i - 2)
                if 1 <= i <= n_tiles:
                    stage_htrans(i - 1)

    nc.compile()
    return nc


def _get_nc(n_tiles):
    if n_tiles not in _cache:
        _cache[n_tiles] = _build_nc(n_tiles)
    return _cache[n_tiles]


def kernel(hidden_states, gate_up_proj, down_proj, num_tokens_per_expert):
    sizes = np.asarray(num_tokens_per_expert)
    offsets = np.concatenate([[0], np.cumsum(sizes)])
    uniform = (
        sizes.shape[0] == E
        and np.all(sizes == T_PER_CORE)
        and hidden_states.shape == (E * T_PER_CORE, H)
    )
    if not uniform:
        # Fallback: host-side numpy (routing metadata other than the
        # compiled uniform case).
        outs = []
        for e in range(sizes.shape[0]):
            xe = hidden_states[offsets[e] : offsets[e + 1]].astype(np.float32)
            merged = xe @ gate_up_proj[e]
            gate, up = merged[:, :I], merged[:, I:]
            he = (gate / (1.0 + np.exp(-gate))) * up
            outs.append(he @ down_proj[e])
        return np.concatenate(outs, axis=0).astype(hidden_states.dtype)

    from concourse.bass_utils import run_bass_kernel_spmd

    nc = _get_nc(T_PER_CORE // 128)
    hs = np.ascontiguousarray(np.asarray(hidden_states, dtype=np.float32))
    w1 = np.ascontiguousarray(np.asarray(gate_up_proj, dtype=np.float32))
    w2 = np.ascontiguousarray(np.asarray(down_proj, dtype=np.float32))
    in_maps = [
        {
            "x": hs[e * T_PER_CORE : (e + 1) * T_PER_CORE],
            "w1": w1[e],
            "w2": w2[e],
        }
        for e in range(N_CORES)
    ]
    res = run_bass_kernel_spmd(nc, in_maps, core_ids=list(range(N_CORES)))
    return np.concatenate([r["out"] for r in res.results], axis=0)



# revision 2
# speedup vs baseline: 1.1361x; 1.1361x over previous
"""Grouped MLP (MoE expert-parallel) Trainium2 kernel.

Problem: hidden_states [131072, 1024] f32, 8 experts each owning a contiguous
16384-token block; per expert: SwiGLU MLP with gate_up [1024, 1024] and
down [512, 1024].

Sharding: expert-parallel — core e computes expert e's token block entirely
locally (no collectives). Inputs are sliced host-side, outputs concatenated.

v2 design (bf16, zero PE transposes):
  - x, w1, w2 are cast to bf16 on the host; out stays f32 (psum precision).
  - x is loaded DIRECTLY transposed via DMA-transpose (xbar), so the PE
    runs only the matmul FLOPs: 12288 cycles per 128 tokens.
  - mm1 is feature-major: lhsT = W1 128x128 block (stationary, FWL),
    rhs = xT [h_k, 512 tokens]; psum out [f_block, 512t].
  - SwiGLU feature-major: gate block g (f 0..511) pairs with up block 4+g;
    silu on ACT, mul on DVE -> h bf16 [i-part, t] == exactly the mm2
    stationary layout (no h transpose).
  - mm2: lhsT = h [i_k, 128t] (stationary), rhs = W2 [i_k, 512f];
    psum [128t, 512] -> DVE copy -> f32 out tile -> DMA store.

bf16 input rounding gives ~2.5e-3 relative error (gate 2e-2).
"""

import numpy as np

E = 8
H = 1024
I = 512
T_PER_CORE = 16384
N_CORES = 8
TT = 512  # token chunk

_cache = {}


def _build_nc(n_tiles):
    """n_tiles = tokens/128 (kept for test.py compat); tokens = n_tiles*128."""
    import concourse.mybir as mybir
    import concourse.tile as tile
    from concourse import bacc

    f32 = mybir.dt.float32
    bf16 = mybir.dt.bfloat16
    act_silu = mybir.ActivationFunctionType.Silu

    n_tok = n_tiles * 128
    assert n_tok % TT == 0
    n_chunks = n_tok // TT
    KH = H // 128  # 8 h-slices
    KI = I // 128  # 4 i-slices

    nc = bacc.Bacc(None, target_bir_lowering=False)
    x = nc.dram_tensor("x", [n_tok, H], bf16, kind="ExternalInput")
    w1 = nc.dram_tensor("w1", [H, 2 * I], bf16, kind="ExternalInput")
    w2 = nc.dram_tensor("w2", [I, H], bf16, kind="ExternalInput")
    out = nc.dram_tensor("out", [n_tok, H], f32, kind="ExternalOutput")

    with tile.TileContext(nc) as tc:
        with (
            tc.tile_pool(name="const", bufs=1) as const,
            tc.tile_pool(name="xtp", bufs=3) as xtp,
            tc.tile_pool(name="sp", bufs=4) as sp,
            tc.tile_pool(name="hp", bufs=2) as hp,
            tc.tile_pool(name="outp", bufs=2) as outp,
            tc.tile_pool(name="ps1", bufs=4, space="PSUM") as ps1p,
            tc.tile_pool(name="ps2", bufs=4, space="PSUM") as ps2p,
        ):
            # Resident weights (bf16, FWL-eligible stationary blocks for mm1).
            w1_sb = const.tile([128, KH, 2 * I], bf16)
            nc.sync.dma_start(w1_sb[:], w1.ap().rearrange("(ho p) f -> p ho f", p=128))
            w2_sb = const.tile([128, KI, H], bf16)
            nc.sync.dma_start(w2_sb[:], w2.ap().rearrange("(io p) f -> p io f", p=128))

            xT_d, h_d = {}, {}

            def stage_load(c):
                # DMA-transpose: x[cTT:(c+1)TT, 128k:128(k+1)] -> xT[:, k, :]
                xT = xtp.tile([128, KH, TT], bf16, tag="xT")
                for k in range(KH):
                    nc.sync.dma_start(
                        xT[:, k, :],
                        x.ap()[c * TT : (c + 1) * TT, k * 128 : (k + 1) * 128],
                        transpose=True,
                    )
                xT_d[c] = xT

            def stage_mm1_swiglu(c):
                xT = xT_d.pop(c)
                h = hp.tile([128, KI, TT], bf16, tag="h")
                for g in range(4):
                    ps_g = ps1p.tile([128, TT], f32, tag="ps1")
                    ps_u = ps1p.tile([128, TT], f32, tag="ps1")
                    for k in range(KH):
                        nc.tensor.matmul(
                            ps_g[:],
                            w1_sb[:, k, g * 128 : (g + 1) * 128],
                            xT[:, k, :],
                            start=(k == 0),
                            stop=(k == KH - 1),
                        )
                    for k in range(KH):
                        nc.tensor.matmul(
                            ps_u[:],
                            w1_sb[:, k, (4 + g) * 128 : (5 + g) * 128],
                            xT[:, k, :],
                            start=(k == 0),
                            stop=(k == KH - 1),
                        )
                    s = sp.tile([128, TT], f32, tag="s")
                    nc.scalar.activation(s[:], ps_g[:], act_silu)
                    nc.vector.tensor_mul(h[:, g, :], s[:], ps_u[:])
                h_d[c] = h

            def stage_mm2_store(c):
                h = h_d.pop(c)
                o = outp.tile([128, TT // 128, H], f32, tag="o")
                for tm in range(TT // 128):
                    for half in range(2):
                        ps2 = ps2p.tile([128, 512], f32, tag="ps2")
                        for k in range(KI):
                            nc.tensor.matmul(
                                ps2[:],
                                h[:, k, tm * 128 : (tm + 1) * 128],
                                w2_sb[:, k, half * 512 : (half + 1) * 512],
                                start=(k == 0),
                                stop=(k == KI - 1),
                            )
                        nc.vector.tensor_copy(
                            o[:, tm, half * 512 : (half + 1) * 512], ps2[:]
                        )
                nc.sync.dma_start(
                    out.ap()[c * TT : (c + 1) * TT, :].rearrange(
                        "(tm p) f -> p tm f", p=128
                    ),
                    o[:],
                )

            for i in range(n_chunks + 1):
                if i < n_chunks:
                    stage_load(i)
                    stage_mm1_swiglu(i)
                if i >= 1:
                    stage_mm2_store(i - 1)

    nc.compile()
    return nc


def _get_nc(n_tiles):
    if n_tiles not in _cache:
        _cache[n_tiles] = _build_nc(n_tiles)
    return _cache[n_tiles]


def _bf16(a):
    import ml_dtypes

    return np.asarray(a, dtype=np.float32).astype(ml_dtypes.bfloat16)


def _make_in_maps(hidden_states, gate_up_proj, down_proj):
    hs = _bf16(hidden_states)
    w1 = _bf16(gate_up_proj)
    w2 = _bf16(down_proj)
    return [
        {
            "x": np.ascontiguousarray(hs[e * T_PER_CORE : (e + 1) * T_PER_CORE]),
            "w1": np.ascontiguousarray(w1[e]),
            "w2": np.ascontiguousarray(w2[e]),
        }
        for e in range(N_CORES)
    ]


def kernel(hidden_states, gate_up_proj, down_proj, num_tokens_per_expert):
    sizes = np.asarray(num_tokens_per_expert)
    offsets = np.concatenate([[0], np.cumsum(sizes)])
    uniform = (
        sizes.shape[0] == E
        and np.all(sizes == T_PER_CORE)
        and hidden_states.shape == (E * T_PER_CORE, H)
    )
    if not uniform:
        # Fallback: host-side numpy (routing metadata other than the
        # compiled uniform case).
        outs = []
        for e in range(sizes.shape[0]):
            xe = hidden_states[offsets[e] : offsets[e + 1]].astype(np.float32)
            merged = xe @ gate_up_proj[e]
            gate, up = merged[:, :I], merged[:, I:]
            he = (gate / (1.0 + np.exp(-gate))) * up
            outs.append(he @ down_proj[e])
        return np.concatenate(outs, axis=0).astype(hidden_states.dtype)

    from concourse.bass_utils import run_bass_kernel_spmd

    nc = _get_nc(T_PER_CORE // 128)
    in_maps = _make_in_maps(hidden_states, gate_up_proj, down_proj)
    res = run_bass_kernel_spmd(nc, in_maps, core_ids=list(range(N_CORES)))
    return np.concatenate([r["out"] for r in res.results], axis=0)


# revision 4
# speedup vs baseline: 6.4989x; 5.7201x over previous
"""Grouped MLP (MoE expert-parallel) Trainium2 kernel.

Problem: hidden_states [131072, 1024] f32, 8 experts each owning a contiguous
16384-token block; per expert: SwiGLU MLP with gate_up [1024, 1024] and
down [512, 1024].

Sharding: expert-parallel — core e computes expert e's token block entirely
locally (no collectives). Inputs are sliced host-side, outputs concatenated.

v2 design (bf16, zero PE transposes):
  - x, w1, w2 are cast to bf16 on the host; out stays f32 (psum precision).
  - x is loaded DIRECTLY transposed via DMA-transpose (xbar), so the PE
    runs only the matmul FLOPs: 12288 cycles per 128 tokens.
  - mm1 is feature-major: lhsT = W1 128x128 block (stationary, FWL),
    rhs = xT [h_k, 512 tokens]; psum out [f_block, 512t].
  - SwiGLU feature-major: gate block g (f 0..511) pairs with up block 4+g;
    silu on ACT, mul on DVE -> h bf16 [i-part, t] == exactly the mm2
    stationary layout (no h transpose).
  - mm2: lhsT = h [i_k, 128t] (stationary), rhs = W2 [i_k, 512f];
    psum [128t, 512] -> DVE copy -> f32 out tile -> DMA store.

bf16 input rounding gives ~4e-3 relative error (gate 2e-2).
"""

import numpy as np

E = 8
H = 1024
I = 512
T_PER_CORE = 16384
N_CORES = 8
TT = 512  # token chunk

_cache = {}


def _build_nc(n_tiles, repeat=1):
    """n_tiles = tokens/128 (kept for test.py compat); tokens = n_tiles*128.

    repeat > 1 wraps the whole kernel (including weight loads) in a
    device-side For_i loop — used only for steady-state timing, so one
    dispatch executes the full kernel `repeat` times back-to-back.
    """
    import concourse.mybir as mybir
    import concourse.tile as tile
    from concourse import bacc
    from contextlib import nullcontext

    f32 = mybir.dt.float32
    bf16 = mybir.dt.bfloat16
    act_silu = mybir.ActivationFunctionType.Silu

    n_tok = n_tiles * 128
    assert n_tok % TT == 0
    n_chunks = n_tok // TT
    KH = H // 128  # 8 h-slices
    KI = I // 128  # 4 i-slices

    nc = bacc.Bacc(None, target_bir_lowering=False)
    x = nc.dram_tensor("x", [n_tok, H], bf16, kind="ExternalInput")
    w1 = nc.dram_tensor("w1", [H, 2 * I], bf16, kind="ExternalInput")
    w2 = nc.dram_tensor("w2", [I, H], bf16, kind="ExternalInput")
    out = nc.dram_tensor("out", [n_tok, H], f32, kind="ExternalOutput")

    with tile.TileContext(nc) as tc:
        with (
            tc.tile_pool(name="wp", bufs=2) as wp,
            tc.tile_pool(name="xtp", bufs=3) as xtp,
            tc.tile_pool(name="sp", bufs=4) as sp,
            tc.tile_pool(name="hp", bufs=2) as hp,
            tc.tile_pool(name="outp", bufs=2) as outp,
            tc.tile_pool(name="ps1", bufs=4, space="PSUM") as ps1p,
            tc.tile_pool(name="ps2", bufs=4, space="PSUM") as ps2p,
        ):
            loop = tc.For_i(0, repeat, 1) if repeat > 1 else nullcontext()
            with loop:
                # Resident weights (bf16, FWL-eligible stationary blocks for
                # mm1); reloaded each repeat so one iteration == the complete
                # kernel.
                w1_sb = wp.tile([128, KH, 2 * I], bf16, tag="w1")
                nc.sync.dma_start(
                    w1_sb[:], w1.ap().rearrange("(ho p) f -> p ho f", p=128)
                )
                w2_sb = wp.tile([128, KI, H], bf16, tag="w2")
                nc.sync.dma_start(
                    w2_sb[:], w2.ap().rearrange("(io p) f -> p io f", p=128)
                )

                xT_d, h_d = {}, {}

                def stage_load(c):
                    # DMA-transpose: x[cTT:(c+1)TT, 128k:128(k+1)] -> xT[:, k, :]
                    xT = xtp.tile([128, KH, TT], bf16, tag="xT")
                    for k in range(KH):
                        nc.sync.dma_start(
                            xT[:, k, :],
                            x.ap()[c * TT : (c + 1) * TT, k * 128 : (k + 1) * 128],
                            transpose=True,
                        )
                    xT_d[c] = xT

                def stage_mm1_swiglu(c):
                    xT = xT_d.pop(c)
                    h = hp.tile([128, KI, TT], bf16, tag="h")
                    for g in range(4):
                        ps_g = ps1p.tile([128, TT], f32, tag="ps1")
                        ps_u = ps1p.tile([128, TT], f32, tag="ps1")
                        for k in range(KH):
                            nc.tensor.matmul(
                                ps_g[:],
                                w1_sb[:, k, g * 128 : (g + 1) * 128],
                                xT[:, k, :],
                                start=(k == 0),
                                stop=(k == KH - 1),
                            )
                        for k in range(KH):
                            nc.tensor.matmul(
                                ps_u[:],
                                w1_sb[:, k, (4 + g) * 128 : (5 + g) * 128],
                                xT[:, k, :],
                                start=(k == 0),
                                stop=(k == KH - 1),
                            )
                        s = sp.tile([128, TT], f32, tag="s")
                        nc.scalar.activation(s[:], ps_g[:], act_silu)
                        nc.vector.tensor_mul(h[:, g, :], s[:], ps_u[:])
                    h_d[c] = h

                def stage_mm2_store(c):
                    h = h_d.pop(c)
                    o = outp.tile([128, TT // 128, H], f32, tag="o")
                    for tm in range(TT // 128):
                        for half in range(2):
                            ps2 = ps2p.tile([128, 512], f32, tag="ps2")
                            for k in range(KI):
                                nc.tensor.matmul(
                                    ps2[:],
                                    h[:, k, tm * 128 : (tm + 1) * 128],
                                    w2_sb[:, k, half * 512 : (half + 1) * 512],
                                    start=(k == 0),
                                    stop=(k == KI - 1),
                                )
                            nc.vector.tensor_copy(
                                o[:, tm, half * 512 : (half + 1) * 512], ps2[:]
                            )
                    nc.sync.dma_start(
                        out.ap()[c * TT : (c + 1) * TT, :].rearrange(
                            "(tm p) f -> p tm f", p=128
                        ),
                        o[:],
                    )

                for i in range(n_chunks + 1):
                    if i < n_chunks:
                        stage_load(i)
                        stage_mm1_swiglu(i)
                    if i >= 1:
                        stage_mm2_store(i - 1)

    nc.compile()
    return nc


def _get_nc(n_tiles):
    if n_tiles not in _cache:
        _cache[n_tiles] = _build_nc(n_tiles)
    return _cache[n_tiles]


def _bf16(a):
    import ml_dtypes

    return np.asarray(a, dtype=np.float32).astype(ml_dtypes.bfloat16)


def _make_in_maps(hidden_states, gate_up_proj, down_proj):
    hs = _bf16(hidden_states)
    w1 = _bf16(gate_up_proj)
    w2 = _bf16(down_proj)
    return [
        {
            "x": np.ascontiguousarray(hs[e * T_PER_CORE : (e + 1) * T_PER_CORE]),
            "w1": np.ascontiguousarray(w1[e]),
            "w2": np.ascontiguousarray(w2[e]),
        }
        for e in range(N_CORES)
    ]


def kernel(hidden_states, gate_up_proj, down_proj, num_tokens_per_expert):
    sizes = np.asarray(num_tokens_per_expert)
    offsets = np.concatenate([[0], np.cumsum(sizes)])
    uniform = (
        sizes.shape[0] == E
        and np.all(sizes == T_PER_CORE)
        and hidden_states.shape == (E * T_PER_CORE, H)
    )
    if not uniform:
        # Fallback: host-side numpy (routing metadata other than the
        # compiled uniform case).
        outs = []
        for e in range(sizes.shape[0]):
            xe = hidden_states[offsets[e] : offsets[e + 1]].astype(np.float32)
            merged = xe @ gate_up_proj[e]
            gate, up = merged[:, :I], merged[:, I:]
            he = (gate / (1.0 + np.exp(-gate))) * up
            outs.append(he @ down_proj[e])
        return np.concatenate(outs, axis=0).astype(hidden_states.dtype)

    from concourse.bass_utils import run_bass_kernel_spmd

    nc = _get_nc(T_PER_CORE // 128)
    in_maps = _make_in_maps(hidden_states, gate_up_proj, down_proj)
    res = run_bass_kernel_spmd(nc, in_maps, core_ids=list(range(N_CORES)))
    return np.concatenate([r["out"] for r in res.results], axis=0)
